# revision 12
# baseline (speedup 1.0000x reference)
"""Trainium2 Bass kernel: causal multi-head attention (B=2, N=2048, C=2048, 16 heads).

Sharding: 16 heads split across 8 cores (2 heads/core, tensor parallel).
Each core computes q/k/v projections for its 2 heads, causal attention,
and its partial out-projection y_c = ctx_c @ wo_c.T. Host sums partials + bo.

v3 (bf16): all matmul operands bfloat16 (same PE rate as fp32r, half the
DMA/SBUF). Structure:
  - qT/kT/vN held in PER-CHUNK tiles so phase-2 score matmuls depend only
    on the chunks they read (tile-granular dependency tracking would
    otherwise stall phase 2 on the last projection chunk)
  - softmax denominators: masked E^T tiles accumulated on DVE into a
    per-block esum (one [P,2,QCW] add per k-tile covering both heads),
    one ones-matmul per (b,qc,h) instead of one per k-tile
  - V transposed to natural layout via the DMA xbar (dma_start_transpose)
  - diagonal k-tiles trimmed to the valid q range; single shared
    [128,2x128] triangular mask multiply
  - out-projection of block i interleaved into block i+1's attention
    k-tile loop; two units reserved for the block boundary to cover the
    DVE recip/ctxmul chain
  - input DMAs batched into fewer, larger dispatches and spread across
    both hardware DGE queues (SP for x, ACT for weights/transposes)
Per-core layout: qT/kT [head_dim partitions, tokens]; vN natural
[token partitions, head_dim]; S^T tiles = K^T.T @ Q^T; E^T = exp(scale*S^T)
(no max subtraction -- scores are ~N(0, 1/9)).
"""

import os
import numpy as np
import ml_dtypes

import concourse.bass as bass
import concourse.tile as tile
from concourse import bacc, mybir
from concourse import bass_utils

F32 = mybir.dt.float32
BF16 = mybir.dt.bfloat16
AF = mybir.ActivationFunctionType

# problem dims (hardcoded per contract)
B = 2
N = 2048
C = 2048
HEADS = 16
HD = 128          # head dim
NCORES = 8
HPC = HEADS // NCORES  # heads per core = 2
E = HPC * HD      # per-core projection width = 256
BN = B * N        # 4096
P = 128
CT = C // P       # 16 contraction tiles
NCH = 512         # n-chunk width for projections
NCHUNKS = BN // NCH   # 8
QCW = 512         # q-chunk width in attention
QCHUNKS = N // QCW    # 4 per batch
KT_PER_B = N // P     # 16 k-tiles per batch
TPC = NCH // P        # token tiles per chunk = 4
SCALE = float(HD) ** -0.5
XP = 4            # x pieces per chunk (4 c-tiles each)

_CACHE = {}


def _build():
    nc = bacc.Bacc(
        "TRN2",
        target_bir_lowering=False,
        debug=False,
        enable_asserts=False,
        num_devices=NCORES,
    )

    xT = nc.dram_tensor("xT", [C, BN], BF16, kind="ExternalInput").ap()
    wqT = nc.dram_tensor("wqT", [C, E], BF16, kind="ExternalInput").ap()
    wkT = nc.dram_tensor("wkT", [C, E], BF16, kind="ExternalInput").ap()
    wvT = nc.dram_tensor("wvT", [C, E], BF16, kind="ExternalInput").ap()
    woT = nc.dram_tensor("woT", [E, C], BF16, kind="ExternalInput").ap()
    bqh = nc.dram_tensor("bqh", [HPC, P], F32, kind="ExternalInput").ap()
    bkh = nc.dram_tensor("bkh", [HPC, P], F32, kind="ExternalInput").ap()
    bvh = nc.dram_tensor("bvh", [HPC, P], F32, kind="ExternalInput").ap()
    masks = nc.dram_tensor("masks", [P, HPC, P], BF16, kind="ExternalInput").ap()
    ones_d = nc.dram_tensor("ones_d", [P, P], BF16, kind="ExternalInput").ap()
    yp = nc.dram_tensor("yp", [BN, C], F32, kind="ExternalOutput").ap()

    with tile.TileContext(nc) as tc:
        with tc.tile_pool(name="persist", bufs=1) as persist:
            # per-chunk persistent activations: fine-grained dependencies
            qTc = [persist.tile([P, HPC, NCH], BF16, tag=f"qT{c}",
                                name=f"qT{c}")
                   for c in range(NCHUNKS)]
            kTc = [persist.tile([P, HPC, NCH], BF16, tag=f"kT{c}",
                                name=f"kT{c}")
                   for c in range(NCHUNKS)]
            vNc = [persist.tile([P, HPC, TPC, HD], BF16, tag=f"vN{c}",
                                name=f"vN{c}")
                   for c in range(NCHUNKS)]
            masks_sb = persist.tile([P, HPC, P], BF16, tag="masks")
            ones_sb = persist.tile([P, P], BF16, tag="ones")
            wo_sb = persist.tile([P, HPC, C], BF16, tag="wo")

            # ---------------- Phase 1: projections ----------------
            with tc.tile_pool(name="p1w", bufs=1) as wpool, \
                 tc.tile_pool(name="p1x", bufs=8) as xpool, \
                 tc.tile_pool(name="p1vt", bufs=2) as vtpool, \
                 tc.tile_pool(name="p1_ps", bufs=8, space="PSUM") as pps:
                wq_sb = wpool.tile([P, CT, E], BF16, tag="wq")
                wk_sb = wpool.tile([P, CT, E], BF16, tag="wk")
                wv_sb = wpool.tile([P, CT, E], BF16, tag="wv")
                bq_sb = wpool.tile([P, HPC], F32, tag="bq")
                bk_sb = wpool.tile([P, HPC], F32, tag="bk")
                bv_sb = wpool.tile([P, HPC], F32, tag="bv")

                # weights + biases dispatched on the ACT hwdge queue, x on
                # the SP queue: two parallel dispatch streams.  Pieces are
                # interleaved in consumption order so the first matmuls
                # start within a few us.
                nc.scalar.dma_start(bq_sb[:], bqh.rearrange("h p -> p h"))
                nc.scalar.dma_start(bk_sb[:], bkh.rearrange("h p -> p h"))
                nc.scalar.dma_start(bv_sb[:], bvh.rearrange("h p -> p h"))
                wsrc = [(wq_sb, wqT), (wk_sb, wkT), (wv_sb, wvT)]
                xTr = xT.rearrange("(t p) n -> p t n", p=P)

                def x_piece(ch, piece):
                    xc = xpool.tile([P, CT // XP, NCH], BF16, tag="xc",
                                    name=f"xc_{ch}_{piece}")
                    nc.sync.dma_start(
                        xc[:], xTr[:, piece * 4:(piece + 1) * 4,
                                   ch * NCH:(ch + 1) * NCH])
                    return xc

                xh0 = []
                for piece in range(XP):
                    for (dst, src) in wsrc:
                        srcr = src.rearrange("(t p) e -> p t e", p=P)
                        nc.scalar.dma_start(
                            dst[:, piece * 4:(piece + 1) * 4, :],
                            srcr[:, piece * 4:(piece + 1) * 4, :],
                        )
                    xh0.append(x_piece(0, piece))
                nc.scalar.dma_start(masks_sb[:], masks)
                nc.scalar.dma_start(ones_sb[:], ones_d)

                for ch in range(NCHUNKS):
                    b = ch // (N // NCH)
                    if ch == 0:
                        xh = xh0
                    else:
                        xh = [x_piece(ch, piece) for piece in range(XP)]
                    if ch == 2:
                        # wo needed from phase 2 on; queue after early x
                        nc.scalar.dma_start(
                            wo_sb[:], woT.rearrange("(h p) f -> p h f", p=P))

                    # 6 accumulators (q/k/v x 2 heads); c-tile outer loop so
                    # each x piece is released after its 4 c-tiles.
                    accs = [pps.tile([P, NCH], F32, tag="pacc",
                                     name=f"pacc_{ch}_{i}")
                            for i in range(3 * HPC)]
                    for ct in range(CT):
                        xq = xh[ct // 4][:, ct % 4, :]
                        for wi, (wsb, _) in enumerate(wsrc):
                            for h in range(HPC):
                                nc.tensor.matmul(
                                    accs[wi * HPC + h][:],
                                    wsb[:, ct, h * HD:(h + 1) * HD],
                                    xq,
                                    start=(ct == 0),
                                    stop=(ct == CT - 1),
                                )

                    for h in range(HPC):
                        nc.scalar.activation(
                            qTc[ch][:, h, :], accs[h][:],
                            AF.Identity, bias=bq_sb[:, h:h + 1], scale=1.0)
                        nc.scalar.activation(
                            kTc[ch][:, h, :], accs[HPC + h][:],
                            AF.Identity, bias=bk_sb[:, h:h + 1], scale=1.0)
                        # v^T with bias, then xbar DMA transpose to V natural
                        vt = vtpool.tile([P, NCH], BF16, tag="vt",
                                         name=f"vt_{ch}_{h}")
                        nc.scalar.activation(
                            vt[:], accs[2 * HPC + h][:],
                            AF.Identity, bias=bv_sb[:, h:h + 1], scale=1.0)
                        nc.scalar.dma_start_transpose(
                            vNc[ch][:, h, :, :], vt[:])

            # ---------------- Phase 2: attention + out-proj ----------------
            with tc.tile_pool(name="p2e", bufs=4) as epool, \
                 tc.tile_pool(name="p2es", bufs=4) as espool, \
                 tc.tile_pool(name="p2ctx", bufs=4) as ctxpool, \
                 tc.tile_pool(name="p2sm", bufs=2) as smpool, \
                 tc.tile_pool(name="p2y", bufs=4) as ystage, \
                 tc.tile_pool(name="p2s_ps", bufs=2, space="PSUM") as spool, \
                 tc.tile_pool(name="p2c_ps", bufs=2, space="PSUM") as cps, \
                 tc.tile_pool(name="p2sb_ps", bufs=2, space="PSUM") as sbps, \
                 tc.tile_pool(name="p2y_ps", bufs=2, space="PSUM") as yps:

                copy_rr = [0]

                def copy_psum(dst, src):
                    """Alternate psum->sbuf copies over DVE/ACT."""
                    eng = copy_rr[0] % 2
                    copy_rr[0] += 1
                    if eng == 0:
                        nc.vector.tensor_copy(dst, src)
                    else:
                        nc.scalar.copy(dst, src)

                def outproj_units(ctx_pair, b, qc):
                    """Out-projection for one (b,qc) block as 8 closures:
                    (nt, fc-pair) -> 4 matmuls + 2 copies + 2 DMAs."""
                    units = []
                    for nt in range(QCW // P):
                        for fcp in range(2):
                            def unit(nt=nt, fcp=fcp):
                                ytiles = [
                                    yps.tile([P, 512], F32, tag="yps",
                                             name=f"yps_{b}_{qc}_{nt}_{fcp}_{j}")
                                    for j in range(2)]
                                for h in range(HPC):
                                    for j in range(2):
                                        fc0 = (2 * fcp + j) * 512
                                        nc.tensor.matmul(
                                            ytiles[j][:],
                                            ctx_pair[h][:, nt * P:(nt + 1) * P],
                                            wo_sb[:, h, fc0:fc0 + 512],
                                            start=(h == 0), stop=(h == HPC - 1),
                                        )
                                row0 = b * N + qc * QCW + nt * P
                                for j in range(2):
                                    fc0 = (2 * fcp + j) * 512
                                    yst = ystage.tile(
                                        [P, 512], F32, tag="yst",
                                        name=f"yst_{b}_{qc}_{nt}_{fcp}_{j}")
                                    copy_psum(yst[:], ytiles[j][:])
                                    nc.sync.dma_start(
                                        yp[row0:row0 + P, fc0:fc0 + 512],
                                        yst[:])
                            units.append(unit)
                    return units

                RESERVE = 2  # units held back to cover the boundary chain
                pending = []
                order = [(b, qc) for b in range(B) for qc in range(QCHUNKS)]
                for (b, qc) in order:
                    nkt = 4 * qc + 4  # causal: k-tiles 0..4qc+3
                    esum = espool.tile([P, HPC, QCW], BF16, tag="esum",
                                       name=f"esum_{b}_{qc}")
                    ctxus = [cps.tile([P, QCW], F32, tag="ctxu",
                                      name=f"ctxu_{b}_{qc}_{h}")
                             for h in range(HPC)]
                    emitted = 0
                    droppable = max(len(pending) - RESERVE, 0)
                    for kt in range(nkt):
                        # drain previous block's out-projection first: these
                        # matmuls depend only on already-final ctx tiles, so
                        # they keep the PE busy while ACT/DVE work on this
                        # block's exp/esum chain.
                        target = ((kt + 1) * droppable + nkt - 1) // nkt
                        while emitted < target:
                            pending[emitted]()
                            emitted += 1
                        # diagonal tiles: columns q < 128a + k are masked;
                        # trim to the valid q range [128a, 512) and mask only
                        # the first 128 columns (same triangular mask for
                        # every diagonal tile).
                        a = kt - 4 * qc
                        q0 = P * a if a > 0 else 0
                        w = QCW - q0
                        ckt = b * (N // NCH) + kt // TPC
                        ko = (kt % TPC) * P
                        et = epool.tile([P, HPC, QCW], BF16, tag="e",
                                        name=f"et_{b}_{qc}_{kt}")
                        for h in range(HPC):
                            sps = spool.tile([P, QCW], F32, tag="s",
                                             name=f"sps_{b}_{qc}_{kt}_{h}")
                            nc.tensor.matmul(
                                sps[:, 0:w],
                                kTc[ckt][:, h, ko:ko + P],
                                qTc[b * (N // NCH) + qc][:, h, q0:QCW],
                                start=True, stop=True,
                            )
                            nc.scalar.activation(
                                et[:, h, 0:w], sps[:, 0:w], AF.Exp,
                                scale=SCALE,
                            )
                        if a >= 0:  # diagonal-of-diagonal causal mask
                            nc.vector.tensor_mul(
                                et[:, :, 0:P], et[:, :, 0:P], masks_sb[:]
                            )
                        if kt == 0:
                            nc.vector.tensor_copy(esum[:], et[:])
                        else:
                            nc.vector.tensor_add(
                                esum[:, :, q0:QCW],
                                esum[:, :, q0:QCW], et[:, :, 0:w])
                        for h in range(HPC):
                            nc.tensor.matmul(
                                ctxus[h][:, q0:QCW],
                                vNc[ckt][:, h, kt % TPC, :],
                                et[:, h, 0:w],
                                start=(kt == 0), stop=(kt == nkt - 1),
                            )
                    while emitted < droppable:
                        pending[emitted]()
                        emitted += 1

                    # block epilogue: denominators, reciprocal, normalize.
                    # The reserved units are emitted between the ones-matmuls
                    # and the next block so the PE has independent work while
                    # the DVE runs recip+normalize.
                    sums = [sbps.tile([P, QCW], F32, tag="sumbc",
                                      name=f"sum_{b}_{qc}_{h}")
                            for h in range(HPC)]
                    for h in range(HPC):
                        nc.tensor.matmul(
                            sums[h][:], ones_sb[:], esum[:, h, :],
                            start=True, stop=True,
                        )
                    ctx_pair = []
                    for h in range(HPC):
                        recip_bc = smpool.tile([P, QCW], F32, tag="recipbc",
                                               name=f"recip_{b}_{qc}_{h}")
                        nc.vector.reciprocal_approx_fast(
                            recip_bc[:], sums[h][:])
                        ctx = ctxpool.tile([P, QCW], BF16, tag="ctx",
                                           name=f"ctx_{b}_{qc}_{h}")
                        nc.vector.tensor_mul(ctx[:], ctxus[h][:], recip_bc[:])
                        ctx_pair.append(ctx)
                    while emitted < len(pending):
                        pending[emitted]()
                        emitted += 1
                    pending = outproj_units(ctx_pair, b, qc)

                for unit in pending:  # flush last block's out-projection
                    unit()

    nc.compile()
    return nc


def _host_prep(x, wq, bq, wk, bk, wv, bv, wo):
    """Build the 8 per-core input maps (bf16 activations/weights)."""
    bf = ml_dtypes.bfloat16
    x = np.asarray(x, dtype=np.float32)
    xT = np.ascontiguousarray(x.reshape(BN, C).T.astype(bf))  # [C, BN]

    kl = np.arange(P)[:, None]
    ql = np.arange(P)[None, :]
    m1 = (ql >= kl).astype(bf)  # triangular diagonal-of-diagonal mask
    m = np.ascontiguousarray(
        np.broadcast_to(m1[:, None, :], (P, HPC, P)).astype(bf))

    in_maps = []
    for c in range(NCORES):
        e0 = c * E
        in_maps.append({
            "xT": xT,
            "wqT": np.ascontiguousarray(
                np.asarray(wq)[e0:e0 + E, :].T.astype(bf)),
            "wkT": np.ascontiguousarray(
                np.asarray(wk)[e0:e0 + E, :].T.astype(bf)),
            "wvT": np.ascontiguousarray(
                np.asarray(wv)[e0:e0 + E, :].T.astype(bf)),
            "woT": np.ascontiguousarray(
                np.asarray(wo)[:, e0:e0 + E].T.astype(bf)),
            "bqh": np.ascontiguousarray(
                np.asarray(bq)[e0:e0 + E].reshape(HPC, P).astype(np.float32)),
            "bkh": np.ascontiguousarray(
                np.asarray(bk)[e0:e0 + E].reshape(HPC, P).astype(np.float32)),
            "bvh": np.ascontiguousarray(
                np.asarray(bv)[e0:e0 + E].reshape(HPC, P).astype(np.float32)),
            "masks": m,
            "ones_d": np.ones((P, P), dtype=bf),
        })
    return in_maps


def _ensure_ntff_hook_module():
    """run_bass_kernel_spmd(trace=True) imports antenv.axon_hooks; provide a
    stub (hook=None -> tracing skipped gracefully) if the module is absent."""
    try:
        import antenv.axon_hooks  # noqa: F401
    except ImportError:
        import sys
        import types
        try:
            import antenv
        except ImportError:
            return
        mod = types.ModuleType("antenv.axon_hooks")
        state = {"hook": None}
        mod.set_axon_ntff_profile_hook = lambda h: state.__setitem__("hook", h)
        mod.get_axon_ntff_profile_hook = lambda: state["hook"]
        sys.modules["antenv.axon_hooks"] = mod
        antenv.axon_hooks = mod


def kernel(**inputs):
    _ensure_ntff_hook_module()
    if "nc" not in _CACHE:
        _CACHE["nc"] = _build()
    nc = _CACHE["nc"]

    in_maps = _host_prep(
        inputs["x"], inputs["wq"], inputs["bq"], inputs["wk"], inputs["bk"],
        inputs["wv"], inputs["bv"], inputs["wo"],
    )

    res = bass_utils.run_bass_kernel_spmd(
        nc, in_maps, core_ids=list(range(NCORES)),
        trace=bool(os.environ.get("BASS_TRACE")),
    )
    _CACHE["last_result"] = res

    y = np.zeros((BN, C), dtype=np.float64)
    for c in range(NCORES):
        y += res.results[c]["yp"].astype(np.float64)
    y += np.asarray(inputs["bo"], dtype=np.float64)
    return y.astype(np.float32).reshape(B, N, C)
